# revision 8
# baseline (speedup 1.0000x reference)
"""GCN (3x GCNConv + BN + final linear) on 8 TRN2 NeuronCores — v2.

Strategy (v2 changes vs v1 in [brackets]):
- Pad N=50000 -> NP=50176 = 392 blocks of 128 nodes. Core c owns 49
  blocks (6272 nodes) and all edges whose destination (col) lies in them.
- norm factorization: dinv[row] folded into edge_attr (host) and the
  gather table rows; dinv[col] applied to aggregated block output.
- [table1 = dinv*(x@W1 + b1 + be1) computed on HOST and passed in; layer 1
  has no AllGather and no on-device node linear.]
- [Gather calls merged per (2-block group, table half): ~50 calls/layer of
  up to ~5k indices instead of ~300 of <=1024 — the 3.2us/call fixed SWDGE
  cost dominated the Pool engine.]
- [edge linear e = ea@We packed 4 chunks per matmul via a block-diagonal
  We (host-built [64, 4*128]), contraction dim 64.]
- [relu(hr + e): DVE adds hr into the e PSUM tile, ACT applies relu ->
  bf16 msg; the per-4-chunk identity-add matmul is gone.]
- Scatter-add via one-hot S matmuls per destination block (unchanged).
- Per layer: BN stats AllReduce, BN+bias folded into next weights on
  device; node linear feature-major; AllGather bf16 table (layers 2,3).
"""

import sys

sys.path.insert(0, "/opt/trn_rl_repo")

import numpy as np
import ml_dtypes

import concourse.bass as bass
import concourse.tile as tile
from concourse import bacc, mybir
from concourse.bass_utils import run_bass_kernel_spmd

# ---------------- constants ----------------
NCORES = 8
D = 128
DE = 16
EPS = 1e-5
P = 128
GRP = 2          # dest blocks per gather group
MAXCH = 8        # max chunks per dma_gather call (ucode caps at 1024 idx)


def configure(n):
    global N, BLOCKS, NP, BPC, NSH, VHALF
    N = n
    BLOCKS = ((N + P - 1) // P + NCORES - 1) // NCORES * NCORES
    NP = BLOCKS * P
    BPC = BLOCKS // NCORES
    NSH = BPC * P
    VHALF = NP // 2


configure(50000)
E = 1_600_000

dt = mybir.dt
AF = mybir.ActivationFunctionType
ALU = mybir.AluOpType


def _bf16(a):
    return np.asarray(a, dtype=np.float32).astype(ml_dtypes.bfloat16)


# ---------------- host-side edge preprocessing ----------------

def _preprocess(edge_index, edge_attr):
    """Sort/pad edges per (core, group, half, dest-block); build packed
    device arrays: eaP [64, TOTWG, P], colrel [P, TOTC], idx16 [128, TOTS].
    """
    row = np.asarray(edge_index[0], dtype=np.int64)
    col = np.asarray(edge_index[1], dtype=np.int64)
    deg = np.bincount(row, minlength=N).astype(np.float32) + 1.0
    dinv = deg ** -0.5
    ea_s = np.asarray(edge_attr, np.float32) * dinv[row][:, None]

    blk = col // P
    order = np.argsort(blk, kind="stable")
    row_s, col_s, blk_s = row[order], col[order], blk[order]
    ea_sorted = ea_s[order]
    starts = np.searchsorted(blk_s, np.arange(BLOCKS))
    ends = np.searchsorted(blk_s, np.arange(BLOCKS), side="right")

    lists = [[None] * BPC for _ in range(NCORES)]
    n_lo = np.zeros((NCORES, BPC), np.int64)
    n_hi = np.zeros((NCORES, BPC), np.int64)
    for g in range(BLOCKS):
        c, b = divmod(g, BPC)
        s, e = starts[g], ends[g]
        r = row_s[s:e]
        rloc = r % NSH
        lo_idx = np.nonzero(rloc < NSH // 2)[0]
        hi_idx = np.nonzero(rloc >= NSH // 2)[0]
        lists[c][b] = (s, lo_idx, hi_idx)
        n_lo[c, b] = len(lo_idx)
        n_hi[c, b] = len(hi_idx)

    m_lo = np.maximum(1, (n_lo.max(axis=0) + P - 1) // P).astype(int)
    m_hi = np.maximum(1, (n_hi.max(axis=0) + P - 1) // P).astype(int)

    groups = [list(range(i, min(i + GRP, BPC))) for i in range(0, BPC, GRP)]
    goff, wgoff, soff, g_m = [], [], [], []
    gc = wc = sc = 0
    for bs in groups:
        mlo = int(sum(m_lo[b] for b in bs))
        mhi = int(sum(m_hi[b] for b in bs))
        m = mlo + mhi
        g_m.append((mlo, mhi))
        goff.append(gc)
        wgoff.append(wc)
        soff.append(sc)
        gc += m
        wc += (m + 3) // 4
        sc += m * 8
    TOTC, TOTWG, TOTS = gc, wc, sc

    per_core = []
    for c in range(NCORES):
        eaP = np.zeros((64, TOTWG, P), np.float32)
        colrel = np.full((P, TOTC), 255, np.uint8)
        idx16 = np.zeros((16, TOTS), np.int16)
        for ng, bs in enumerate(groups):
            mlo, mhi = g_m[ng]
            # chunk-granular padded streams, pair order: lo segs then hi segs
            rows_all = []
            cols_all = []
            ea_all = []
            for half, base in ((0, 0), (1, NSH // 2)):
                for b in bs:
                    s, lo_idx, hi_idx = lists[c][b]
                    sub = lo_idx if half == 0 else hi_idx
                    mch = int(m_lo[b] if half == 0 else m_hi[b])
                    g_sz = mch * P
                    rows_h = np.zeros(g_sz, np.int64)
                    rsub = row_s[s + sub]
                    rows_h[: len(sub)] = ((rsub // NSH) * (NSH // 2)
                                          + (rsub % NSH) - base)
                    cols_h = np.full(g_sz, 255, np.int64)
                    cols_h[: len(sub)] = col_s[s + sub] - (c * BPC + b) * P
                    ea_h = np.zeros((g_sz, DE), np.float32)
                    ea_h[: len(sub)] = ea_sorted[s + sub]
                    rows_all.append(rows_h)
                    cols_all.append(cols_h)
                    ea_all.append(ea_h)
            rows_g = np.concatenate(rows_all)          # [(mlo+mhi)*128]
            cols_g = np.concatenate(cols_all)
            ea_g = np.concatenate(ea_all)              # [(mlo+mhi)*128, 16]
            m = mlo + mhi
            # colrel + eaP in pair-chunk order
            ii = np.arange(m * P)
            pp, qq = ii % P, ii // P
            colrel[pp, goff[ng] + qq] = cols_g
            ci = qq % 4
            wg = qq // 4
            for f in range(DE):
                eaP[ci * DE + f, wgoff[ng] + wg, pp] = ea_g[:, f]
            # idx16: two calls, local ii within each call
            for half, cnt, off in ((0, mlo, 0), (1, mhi, mlo)):
                jj = np.arange(cnt * P)
                src = rows_g[off * P: off * P + cnt * P]
                idx16[jj % 16, soff[ng] + off * 8 + jj // 16] = src
        per_core.append(
            dict(
                eaP=_bf16(eaP),
                colrel=colrel,
                idx16=np.tile(idx16, (8, 1)),
            )
        )

    sched = dict(
        groups=groups, g_m=g_m, goff=goff, wgoff=wgoff, soff=soff,
        m_lo=[int(v) for v in m_lo], m_hi=[int(v) for v in m_hi],
        TOTC=TOTC, TOTWG=TOTWG, TOTS=TOTS,
    )
    return per_core, sched, dinv


# ---------------- device program ----------------

def _build(sched):
    nc = bacc.Bacc(None, target_bir_lowering=False, debug=False,
                   num_swdge_queues=4)
    TOTC, TOTWG, TOTS = sched["TOTC"], sched["TOTWG"], sched["TOTS"]
    groups, g_m = sched["groups"], sched["g_m"]
    goff, wgoff, soff = sched["goff"], sched["wgoff"], sched["soff"]
    m_lo, m_hi = sched["m_lo"], sched["m_hi"]

    decl = nc.declare_dram_parameter
    tab1_d = decl("tab1", [NP, D], dt.bfloat16, isOutput=False)
    eaP_d = decl("eaP", [64, TOTWG, P], dt.bfloat16, isOutput=False)
    colrel_d = decl("colrel", [P, TOTC], dt.uint8, isOutput=False)
    idx_d = decl("idx16", [P, TOTS], dt.int16, isOutput=False)
    dinv_d = decl("dinvt", [P, NSH], dt.bfloat16, isOutput=False)
    iota_d = decl("iota_u8", [P, P], dt.uint8, isOutput=False)
    ident_d = decl("ident", [P, P], dt.bfloat16, isOutput=False)
    W_d = [decl(f"W{k}", [D, D], dt.bfloat16, isOutput=False) for k in (2, 3)]
    Wf_d = [decl(f"Wf{k}", [D, D], dt.float32, isOutput=False) for k in (2, 3)]
    Wl_d = decl("Wlin", [D, D], dt.bfloat16, isOutput=False)
    Wlf_d = decl("Wlinf", [D, D], dt.float32, isOutput=False)
    webd_d = [decl(f"WeBD{k}", [64, 4 * D], dt.bfloat16, isOutput=False)
              for k in (1, 2, 3)]
    brow_d = [decl(f"brow{k}", [1, D], dt.float32, isOutput=False) for k in (2, 3)]
    blrow_d = decl("blrow", [1, D], dt.float32, isOutput=False)
    g_d = [decl(f"g{k}", [D, 1], dt.float32, isOutput=False) for k in (1, 2, 3)]
    bt_d = [decl(f"bt{k}", [D, 1], dt.float32, isOutput=False) for k in (1, 2, 3)]
    outT = decl("outT", [P, NSH], dt.float32, isOutput=True)

    import os
    SKIP_CC = os.environ.get("KSKIP_CC") == "1"
    rg = [list(range(NCORES))]

    with tile.TileContext(nc) as tc:
        import contextlib
        with contextlib.ExitStack() as ctx:
            ek = ctx.enter_context
            const = ek(tc.tile_pool(name="const", bufs=1))
            edge_ea = ek(tc.tile_pool(name="edge_ea", bufs=3))
            edge_idx = ek(tc.tile_pool(name="edge_idx", bufs=3))
            edge_hr = ek(tc.tile_pool(name="edge_hr", bufs=3))
            edge_msg = ek(tc.tile_pool(name="edge_msg", bufs=2))
            edge_S = ek(tc.tile_pool(name="edge_S", bufs=2))
            small = ek(tc.tile_pool(name="small", bufs=4))
            trp = ek(tc.tile_pool(name="trp", bufs=3))
            
            ps_mp = ek(tc.tile_pool(name="ps_mp", bufs=3, space="PSUM"))
            ps_conv = ek(tc.tile_pool(name="ps_conv", bufs=2, space="PSUM"))
            ps_misc = ek(tc.tile_pool(name="ps_misc", bufs=1, space="PSUM"))
            dram = ek(tc.tile_pool(name="dram", bufs=2, space="DRAM"))
            dram_tab = ek(tc.tile_pool(name="dram_tab", bufs=1, space="DRAM"))

            def ld(pool, shape, dty, src, name):
                t = pool.tile(shape, dty, name=name)
                nc.sync.dma_start(out=t[:], in_=src[...])
                return t

            dinv_t = ld(const, [P, NSH], dt.bfloat16, dinv_d, 'dinv_t')
            iota_t = ld(const, [P, P], dt.uint8, iota_d, 'iota_t')
            ident_t = ld(const, [P, P], dt.bfloat16, ident_d, 'ident_t')
            colrel_t = ld(const, [P, TOTC], dt.uint8, colrel_d, 'colrel_t')
            W_t = [ld(const, [D, D], dt.bfloat16, W_d[i], f'W_t{i}') for i in range(2)]
            Wf_t = [ld(const, [D, D], dt.float32, Wf_d[i], f'Wf_t{i}') for i in range(2)]
            Wl_t = ld(const, [D, D], dt.bfloat16, Wl_d, 'Wl_t')
            Wlf_t = ld(const, [D, D], dt.float32, Wlf_d, 'Wlf_t')
            webd_t = [ld(const, [64, 4, D], dt.bfloat16, webd_d[i], f'webd_t{i}')
                      for i in range(3)]
            brow_t = [ld(const, [1, D], dt.float32, brow_d[i], f'brow_t{i}')
                      for i in range(2)]
            blrow_t = ld(const, [1, D], dt.float32, blrow_d, 'blrow_t')
            g_t = [ld(const, [D, 1], dt.float32, g_d[i], f'g_t{i}') for i in range(3)]
            bt_t = [ld(const, [D, 1], dt.float32, bt_d[i], f'bt_t{i}') for i in range(3)]

            t_T = [const.tile([P, NSH], dt.bfloat16, name=f't_T{i}') for i in range(2)]
            eps_t = const.tile([P, 1], dt.float32, name='eps_t')
            nc.vector.memset(eps_t[:], EPS)

            # two persistent DRAM gather tables (layer k uses T[k%2])
            tabs = [dram_tab.tile([NP, D], dt.bfloat16, name=f'tab{i}')
                    for i in range(2)]
            nc.sync.dma_start(out=tabs[0][:, :], in_=tab1_d[...])

            col_chunks = [(o, min(512, NSH - o)) for o in range(0, NSH, 512)]

            qrr = [0]

            def gather_range(hr_t, c0, nch, tab_ap, idx_t, scol0):
                """Gather nch chunks into hr_t[:, c0:c0+nch, :] in <=MAXCH
                chunk calls, round-robin across SWDGE queues."""
                done = 0
                while done < nch:
                    w = min(MAXCH, nch - done)
                    nc.gpsimd.dma_gather(
                        out_ap=hr_t[:, c0 + done:c0 + done + w, :],
                        in_ap=tab_ap,
                        idxs_ap=idx_t[:, scol0 + done * 8:scol0 + (done + w) * 8],
                        num_idxs=w * P, num_idxs_reg=w * P, elem_size=D,
                        queue_num=qrr[0] % 4,
                    )
                    qrr[0] += 1
                    done += w

            for k in range(3):
                table = tabs[k % 2]
                sums_t = small.tile([P, BPC], dt.float32)
                sqs_t = small.tile([P, BPC], dt.float32)
                tnew = t_T[k % 2]

                for ng, bs in enumerate(groups):
                    mlo, mhi = g_m[ng]
                    m = mlo + mhi
                    ngw = (m + 3) // 4
                    ea_t = edge_ea.tile([64, ngw, P], dt.bfloat16, name='ea_g')
                    nc.sync.dma_start(out=ea_t[:],
                                      in_=eaP_d[:, wgoff[ng]:wgoff[ng] + ngw, :])
                    idx_t = edge_idx.tile([P, m * 8], dt.int16, name='idx_g')
                    nc.sync.dma_start(out=idx_t[:],
                                      in_=idx_d[:, soff[ng]:soff[ng] + m * 8])
                    hr_t = edge_hr.tile([P, m, D], dt.bfloat16, name='hr_g')
                    gather_range(hr_t, 0, mlo, table[:VHALF, :], idx_t, 0)
                    gather_range(hr_t, mlo, mhi, table[VHALF:, :], idx_t, mlo * 8)

                    # S indicator [P, m, P]
                    S_t = edge_S.tile([P, m, P], dt.bfloat16, name='S_g')
                    iota_b = bass.AP(tensor=iota_t.tensor, offset=iota_t[:].offset,
                                     ap=[iota_t[:].ap[0], [0, m], iota_t[:].ap[1]])
                    cr = colrel_t[:, goff[ng]:goff[ng] + m]
                    cr_b = bass.AP(tensor=colrel_t.tensor, offset=cr.offset,
                                   ap=[cr.ap[0], cr.ap[1], [0, P]])
                    nc.vector.tensor_tensor(out=S_t[:], in0=iota_b, in1=cr_b,
                                            op=ALU.is_equal)

                    # messages: e = ea@We (block-diag, 4 chunks/shot),
                    # += hr (DVE), relu -> bf16 (ACT)
                    msg_t = edge_msg.tile([P, m, D], dt.bfloat16, name='msg_g')
                    for g in range(ngw):
                        c0 = g * 4
                        cw = min(4, m - c0)
                        mp = ps_mp.tile([P, 4, D], dt.float32, space="PSUM")
                        nc.tensor.matmul(
                            out=mp[:, :cw, :].rearrange("p j d -> p (j d)"),
                            lhsT=ea_t[:16 * cw, g, :],
                            rhs=webd_t[k][:16 * cw, :cw, :].rearrange(
                                "p j d -> p (j d)"),
                            start=True, stop=True)
                        nc.vector.tensor_tensor(
                            out=mp[:, :cw, :], in0=mp[:, :cw, :],
                            in1=hr_t[:, c0:c0 + cw, :], op=ALU.add)
                        nc.scalar.activation(
                            out=msg_t[:, c0:c0 + cw, :].rearrange("p j d -> p (j d)"),
                            in_=mp[:, :cw, :].rearrange("p j d -> p (j d)"),
                            func=AF.Relu)

                    # scatter per dest block
                    lo_base = 0
                    hi_base = mlo
                    for bi, b in enumerate(bs):
                        ranges = []
                        ranges.append((lo_base, m_lo[b]))
                        lo_base += m_lo[b]
                        ranges.append((hi_base, m_hi[b]))
                        hi_base += m_hi[b]
                        js = [start + j for start, cnt in ranges for j in range(cnt)]
                        cp = ps_conv.tile([P, P], dt.float32, space="PSUM",
                                          padded_shape=[P, 512])
                        for ji, j in enumerate(js):
                            nc.tensor.matmul(out=cp[:], lhsT=msg_t[:, j, :],
                                             rhs=S_t[:, j, :],
                                             start=(ji == 0), stop=(ji == len(js) - 1))
                        sl = slice(b * P, (b + 1) * P)
                        pre = trp.tile([P, P], dt.float32)
                        nc.vector.tensor_tensor(out=pre[:], in0=cp[:],
                                                in1=dinv_t[:, sl], op=ALU.mult)
                        nc.scalar.activation(out=tnew[:, sl], in_=pre[:], func=AF.Relu,
                                             accum_out=sums_t[:, b:b + 1])
                        sq_scr = trp.tile([P, P], dt.bfloat16)
                        nc.scalar.activation(out=sq_scr[:], in_=tnew[:, sl],
                                             func=AF.Square,
                                             accum_out=sqs_t[:, b:b + 1])

                # ---- BN stats + fold coefficients ----
                st = small.tile([P, 2], dt.float32)
                nc.vector.tensor_reduce(out=st[:, 0:1], in_=sums_t[:],
                                        axis=mybir.AxisListType.X, op=ALU.add)
                nc.vector.tensor_reduce(out=st[:, 1:2], in_=sqs_t[:],
                                        axis=mybir.AxisListType.X, op=ALU.add)
                st_in = dram.tile([P, 2], dt.float32)
                st_out = dram.tile([P, 2], dt.float32)
                nc.sync.dma_start(out=st_in[:], in_=st[:])
                if not SKIP_CC:
                    nc.gpsimd.collective_compute(
                        "AllReduce", ALU.add, replica_groups=rg,
                        ins=[st_in[:].opt()], outs=[st_out[:].opt()],
                    )
                else:
                    nc.sync.dma_start(out=st_out[:, :], in_=st_in[:, :])
                stg = small.tile([P, 2], dt.float32)
                nc.sync.dma_start(out=stg[:], in_=st_out[:])
                mu = small.tile([P, 1], dt.float32)
                nc.vector.tensor_scalar(out=mu[:], in0=stg[:, 0:1], scalar1=1.0 / N,
                                        scalar2=None, op0=ALU.mult)
                ex2 = small.tile([P, 1], dt.float32)
                nc.vector.tensor_scalar(out=ex2[:], in0=stg[:, 1:2], scalar1=1.0 / N,
                                        scalar2=None, op0=ALU.mult)
                var = small.tile([P, 1], dt.float32)
                nc.vector.tensor_tensor(out=var[:], in0=mu[:], in1=mu[:], op=ALU.mult)
                nc.vector.tensor_tensor(out=var[:], in0=ex2[:], in1=var[:],
                                        op=ALU.subtract)
                sd = small.tile([P, 1], dt.float32)
                nc.scalar.activation(out=sd[:], in_=var[:], func=AF.Sqrt, bias=eps_t[:])
                rs = small.tile([P, 1], dt.float32)
                nc.vector.reciprocal(out=rs[:], in_=sd[:])
                a_t = small.tile([P, 1], dt.float32)
                nc.vector.tensor_tensor(out=a_t[:], in0=rs[:], in1=g_t[k][:],
                                        op=ALU.mult)
                c_t = small.tile([P, 1], dt.float32)
                nc.vector.tensor_tensor(out=c_t[:], in0=mu[:], in1=a_t[:], op=ALU.mult)
                nc.vector.tensor_tensor(out=c_t[:], in0=bt_t[k][:], in1=c_t[:],
                                        op=ALU.subtract)

                if k < 2:
                    # fold BN into W_{k+1}; node linear; shard; AllGather
                    Wp_t = small.tile([D, D], dt.bfloat16)
                    nc.scalar.activation(out=Wp_t[:], in_=W_t[k][:], func=AF.Identity,
                                         scale=a_t[:])
                    pb = ps_misc.tile([1, D], dt.float32, space="PSUM",
                                      padded_shape=[1, 512], name='pb')
                    nc.tensor.matmul(out=pb[:], lhsT=c_t[:], rhs=Wf_t[k][:],
                                     start=True, stop=True)
                    bprow = small.tile([1, D], dt.float32)
                    nc.vector.tensor_tensor(out=bprow[:], in0=pb[:], in1=brow_t[k][:],
                                            op=ALU.add)
                    bp_bounce = dram.tile([1, D], dt.float32, name='bp_bounce')
                    nc.sync.dma_start(out=bp_bounce[:], in_=bprow[:])
                    bp_t = small.tile([D, 1], dt.float32)
                    nc.sync.dma_start(out=bp_t[:], in_=bp_bounce[0, :, None])

                    shard = dram.tile([NSH, D], dt.bfloat16)
                    for (o, w) in col_chunks:
                        pp = ps_misc.tile([P, w], dt.float32, space="PSUM",
                                          padded_shape=[P, 512], name='pp')
                        nc.tensor.matmul(out=pp[:], lhsT=Wp_t[:], rhs=tnew[:, o:o + w],
                                         start=True, stop=True)
                        tmp = trp.tile([P, w], dt.bfloat16, name='hltmp')
                        nc.scalar.activation(out=tmp[:], in_=pp[:], func=AF.Identity,
                                             bias=bp_t[:])
                        hl = trp.tile([P, w], dt.bfloat16, name='hlc')
                        nc.vector.tensor_tensor(out=hl[:], in0=tmp[:],
                                                in1=dinv_t[:, o:o + w], op=ALU.mult)
                        for t in range(w // P):
                            ptr = ps_misc.tile([P, P], dt.bfloat16, space="PSUM",
                                               padded_shape=[P, 1024], name='ptr')
                            nc.tensor.transpose(out=ptr[:],
                                                in_=hl[:, t * P:(t + 1) * P],
                                                identity=ident_t[:])
                            sb = trp.tile([P, P], dt.bfloat16, name='shsb')
                            nc.scalar.activation(out=sb[:], in_=ptr[:], func=AF.Copy)
                            nc.sync.dma_start(
                                out=shard[o + t * P:o + (t + 1) * P, :], in_=sb[:])
                    tnext = tabs[(k + 1) % 2]
                    if not SKIP_CC:
                        nc.gpsimd.collective_compute(
                            "AllGather", ALU.bypass, replica_groups=rg,
                            ins=[shard[:NSH // 2, :].opt()],
                            outs=[tnext[:VHALF, :].opt()],
                        )
                        nc.gpsimd.collective_compute(
                            "AllGather", ALU.bypass, replica_groups=rg,
                            ins=[shard[NSH // 2:, :].opt()],
                            outs=[tnext[VHALF:, :].opt()],
                        )
                    else:
                        nc.sync.dma_start(out=tnext[:NSH, :], in_=shard[:, :])
                else:
                    # final linear: out^T = Wl'.T @ t3 + bl'
                    Wlp = small.tile([D, D], dt.bfloat16)
                    nc.scalar.activation(out=Wlp[:], in_=Wl_t[:], func=AF.Identity,
                                         scale=a_t[:])
                    pb = ps_misc.tile([1, D], dt.float32, space="PSUM",
                                      padded_shape=[1, 512], name='pb')
                    nc.tensor.matmul(out=pb[:], lhsT=c_t[:], rhs=Wlf_t[:],
                                     start=True, stop=True)
                    blp_row = small.tile([1, D], dt.float32)
                    nc.vector.tensor_tensor(out=blp_row[:], in0=pb[:], in1=blrow_t[:],
                                            op=ALU.add)
                    blp_bounce = dram.tile([1, D], dt.float32, name='blp_bounce')
                    nc.sync.dma_start(out=blp_bounce[:], in_=blp_row[:])
                    blp = small.tile([D, 1], dt.float32)
                    nc.sync.dma_start(out=blp[:], in_=blp_bounce[0, :, None])
                    for (o, w) in col_chunks:
                        pp = ps_misc.tile([P, w], dt.float32, space="PSUM",
                                          padded_shape=[P, 512], name='pp')
                        nc.tensor.matmul(out=pp[:], lhsT=Wlp[:], rhs=tnew[:, o:o + w],
                                         start=True, stop=True)
                        ot = trp.tile([P, w], dt.float32)
                        nc.scalar.activation(out=ot[:], in_=pp[:], func=AF.Identity,
                                             bias=blp[:])
                        nc.sync.dma_start(out=outT[:, o:o + w], in_=ot[:])

    nc.finalize()
    return nc


# ---------------- public entry point ----------------

_CACHE = {}
LAST_EXEC_NS = None


def _make_in_maps(inputs, per_core, dinv):
    x = np.asarray(inputs["x"], np.float32)
    W1 = np.asarray(inputs["W1"], np.float32)
    b_tot1 = np.asarray(inputs["b1"], np.float32) + np.asarray(inputs["be1"],
                                                              np.float32)
    dinv_pad = np.zeros(NP, np.float32)
    dinv_pad[:N] = dinv
    tab1 = np.zeros((NP, D), np.float32)
    tab1[:N] = (x @ W1 + b_tot1) * dinv[:, None]
    # rank-interleaved half layout: [all ranks' first NSH/2 rows | rest]
    t3 = tab1.reshape(NCORES, NSH, D)
    tab1 = np.concatenate([t3[:, :NSH // 2].reshape(-1, D),
                           t3[:, NSH // 2:].reshape(-1, D)])
    tab1 = _bf16(tab1)

    Ws = {k: np.asarray(inputs[k], np.float32) for k in
          ("W2", "W3", "Wl", "We1", "We2", "We3")}
    bt_tot = {k: np.asarray(inputs[f"b{k}"], np.float32) +
                 np.asarray(inputs[f"be{k}"], np.float32) for k in (2, 3)}

    webd = {}
    for k in (1, 2, 3):
        bd = np.zeros((64, 4, D), np.float32)
        We = np.asarray(inputs[f"We{k}"], np.float32)
        for ci in range(4):
            bd[ci * DE:(ci + 1) * DE, ci, :] = We
        webd[k] = _bf16(bd.reshape(64, 4 * D))

    in_maps = []
    for c in range(NCORES):
        sl = slice(c * NSH, (c + 1) * NSH)
        im = dict(per_core[c])
        im["tab1"] = tab1
        im["dinvt"] = _bf16(np.tile(dinv_pad[sl][None, :], (P, 1)))
        im["iota_u8"] = np.tile(np.arange(P, dtype=np.uint8)[None, :], (P, 1))
        im["ident"] = _bf16(np.eye(P))
        for k in (1, 2, 3):
            im[f"WeBD{k}"] = webd[k]
            im[f"g{k}"] = np.asarray(inputs[f"g{k}"], np.float32).reshape(D, 1)
            im[f"bt{k}"] = np.asarray(inputs[f"bt{k}"], np.float32).reshape(D, 1)
        im["W2"] = _bf16(Ws["W2"])
        im["W3"] = _bf16(Ws["W3"])
        im["Wf2"] = Ws["W2"]
        im["Wf3"] = Ws["W3"]
        im["Wlin"] = _bf16(Ws["Wl"])
        im["Wlinf"] = Ws["Wl"]
        im["brow2"] = bt_tot[2].reshape(1, D)
        im["brow3"] = bt_tot[3].reshape(1, D)
        im["blrow"] = np.asarray(inputs["bl"], np.float32).reshape(1, D)
        in_maps.append(im)
    return in_maps


def kernel(**inputs):
    edge_attr = np.asarray(inputs["edge_attr"], np.float32)
    edge_index = np.asarray(inputs["edge_index"])

    per_core, sched, dinv = _preprocess(edge_index, edge_attr)
    in_maps = _make_in_maps(inputs, per_core, dinv)

    key = ("k2", sched["TOTC"], sched["TOTWG"], sched["TOTS"],
           tuple(sched["m_lo"]), tuple(sched["m_hi"]))
    if key not in _CACHE:
        _CACHE[key] = _build(sched)
    nc = _CACHE[key]

    import os
    trace = os.environ.get("KPROF") == "1"
    r = run_bass_kernel_spmd(nc, in_maps, core_ids=list(range(NCORES)), trace=trace)
    if trace:
        print(f"HW exec time: {r.exec_time_ns} ns", flush=True)
        global LAST_EXEC_NS
        LAST_EXEC_NS = r.exec_time_ns
    res = r.results
    outT_full = np.concatenate([res[c]["outT"] for c in range(NCORES)], axis=1)
    return np.ascontiguousarray(outT_full.T[:N]).astype(np.float32)
